# revision 34
# baseline (speedup 1.0000x reference)
"""Trainium2 Bass kernel for nn_AttnResBlock (B=16, C=512, A=64, L=1024).

Data-parallel over batch: 8 cores x 2 batches each, weights replicated.
BatchNorm (training mode, stats over (B, L)) needs global batch stats ->
two tiny [128, 8] f32 AllReduces; a same-shape warmup collective at kernel
start absorbs the ~50us first-collective setup under phase A.

v2 schedule (vs baseline):
  - kq projections in bf16 (host-cast copy of x); x stays f32 for the
    residual only. xT host-supplied in bf16 (feeds xTs directly).
  - fine-grained startup DMAs: wkq + xb(b0) first -> first matmul ~6us.
  - phase A interleaved across the 2 local batches to kill the
    batch-transition bubble: kq(b0,b1) | scores b0 | attnout b0 x
    scores b1 | proj b0 | attnout b1 | proj b1.
  - BN stats reduced eagerly per channel-tile; sum-of-squares on ACT
    (idle during proj/conv) so the AllReduce fires ~1us after the last
    proj/conv matmul.
  - w1 loaded during phase A; w2 on the gpsimd queue right after AR1's
    result DMA (never contends with a collective).
  - h = relu(bn(x)) written in half-L chunks ordered to match the conv
    matmul groups, so conv starts ~2us after the AllReduce lands.
  - conv accumulation groups un-interleaved (12 consecutive matmuls per
    PSUM bank); outputs stored per 512-chunk to shrink the tail.
"""
import numpy as np

P = 128
B, C, A, L = 16, 512, 64, 1024
NCORES = 8
BL = B // NCORES          # local batches per core
CT = C // P               # 4 channel tiles
LT = L // P               # 8 length tiles
MC = L // 512             # 2 moving chunks
EPS = 1e-5
SM_SCALE = 2.0 / L        # softmax scale: scores/(L/2)
H1 = 514                  # first h chunk covers cols 0:514 (taps for hc=0)

_CACHE = {}


def _build():
    import concourse.bass as bass
    import concourse.mybir as mybir
    from concourse import bacc
    from concourse.tile import TileContext

    f32 = mybir.dt.float32
    bf16 = mybir.dt.bfloat16
    f8 = mybir.dt.float8e4
    DR = mybir.MatmulPerfMode.DoubleRow
    AF = mybir.ActivationFunctionType
    ALU = mybir.AluOpType

    nc = bacc.Bacc(num_devices=NCORES)

    x_ext = nc.declare_dram_parameter("x", [BL, C, L], f32, isOutput=False)
    xb_ext = nc.declare_dram_parameter("xb", [BL, C, L], bf16, isOutput=False)
    xT_ext = nc.declare_dram_parameter("xT", [BL, L, C], bf16, isOutput=False)
    wkq_ext = nc.declare_dram_parameter("wkq", [P, CT * 2 * A], bf16, isOutput=False)
    wp_ext = nc.declare_dram_parameter("wp", [P, CT * C], f8, isOutput=False)
    w1_ext = nc.declare_dram_parameter("w1", [P, 3 * CT * C], bf16, isOutput=False)
    w2_ext = nc.declare_dram_parameter("w2", [P, 3 * CT * C], bf16, isOutput=False)
    # per-channel vectors packed [P, CT] each: bp b1 b2 g1 be1 g2 be2, then bkq
    pvec_ext = nc.declare_dram_parameter("pvec", [P, 7 * CT + 1], f32, isOutput=False)
    out_ext = nc.declare_dram_parameter("out", [BL, C, L], f32, isOutput=True)

    cc0_in = nc.dram_tensor("cc0_in", [P, 2 * CT], f32)
    cc0_out = nc.dram_tensor("cc0_out", [P, 2 * CT], f32, addr_space="Shared")
    cc1_in = nc.dram_tensor("cc1_in", [P, 2 * CT], f32)
    cc1_out = nc.dram_tensor("cc1_out", [P, 2 * CT], f32, addr_space="Shared")
    cc2_in = nc.dram_tensor("cc2_in", [P, 2 * CT], f32)
    cc2_out = nc.dram_tensor("cc2_out", [P, 2 * CT], f32, addr_space="Shared")

    rg = [list(range(NCORES))]

    with TileContext(nc) as tc:
        with tc.tile_pool(name="persist", bufs=1) as pers, \
             tc.tile_pool(name="small", bufs=1) as small, \
             tc.tile_pool(name="ostage", bufs=4) as ostage, \
             tc.tile_pool(name="psum", bufs=7, space="PSUM") as psum, \
             tc.tile_pool(name="psumf", bufs=1, space="PSUM") as psumf:

            x2_sb = pers.tile([P, BL, CT, L], f32)
            w1_sb = pers.tile([P, 3 * CT, C], bf16, tag="w1")

            # warmup collective: absorbs the first-collective setup cost
            # (~50us barrier+init) under phase A
            nc.gpsimd.collective_compute(
                "AllReduce", mybir.AluOpType.add, replica_groups=rg,
                ins=[cc0_in[:].opt()], outs=[cc0_out[:].opt()])

            pvec_sb = small.tile([P, 7 * CT + 1], f32, tag="pvec")
            nc.gpsimd.dma_start(out=pvec_sb[:], in_=pvec_ext[:])
            bp_sb = pvec_sb[:, 0 * CT:1 * CT]
            b1_sb = pvec_sb[:, 1 * CT:2 * CT]
            b2_sb = pvec_sb[:, 2 * CT:3 * CT]
            g1_sb = pvec_sb[:, 3 * CT:4 * CT]
            be1_sb = pvec_sb[:, 4 * CT:5 * CT]
            g2_sb = pvec_sb[:, 5 * CT:6 * CT]
            be2_sb = pvec_sb[:, 6 * CT:7 * CT]
            bkq_sb = pvec_sb[:, 7 * CT:7 * CT + 1]   # [bk; bq]

            # stat layout [P, oc, (sum, sumsq)] so each oc's pair is adjacent
            # (enables eager per-oc DMA of AllReduce input)
            ccin1_sb = small.tile([P, CT, 2], f32, tag="ccin1")
            ccout1_sb = small.tile([P, CT, 2], f32, tag="ccout1")
            ccin2_sb = small.tile([P, CT, 2], f32, tag="ccin2")
            ccout2_sb = small.tile([P, CT, 2], f32, tag="ccout2")
            # per-chunk stat accumulators: [P, ct, 2*b+chunk]
            m1a = small.tile([P, CT, 2 * BL], f32, tag="m1a")   # sum(x2)
            m2a = small.tile([P, CT, 2 * BL], f32, tag="m2a")   # sum(x2^2)
            n1a = small.tile([P, CT, 2 * BL], f32, tag="n1a")   # sum(h2)
            n2a = small.tile([P, CT, 2 * BL], f32, tag="n2a")   # sum(h2^2)
            scale1 = small.tile([P, CT], f32, tag="scale1")
            bias1 = small.tile([P, CT], f32, tag="bias1")
            scale2 = small.tile([P, CT], f32, tag="scale2")
            bias2 = small.tile([P, CT], f32, tag="bias2")
            eps_sb = small.tile([P, 1], f32, tag="eps")
            nc.vector.memset(eps_sb[:], EPS)
            # scratch for post-AllReduce PE re-warm fillers (written after
            # bn_post so the fillers become runnable just before the convs)
            scrw = small.tile([P, 640], bf16, tag="scrw")
            nc.vector.memset(scrw[:], 0.0)
            fillps = psumf.tile([P, 512], f32, tag="fill")

            def filler(n):
                # dummy matmuls with no data deps: keep HAM at 8/8 through
                # dependency bubbles (a >=3.4us PE idle re-throttles the PE
                # clock to 1.2GHz for 10-30us)
                for _ in range(n):
                    nc.tensor.matmul(out=fillps[:], lhsT=scrw[:, 0:128],
                                     rhs=scrw[:, 128:640], start=True,
                                     stop=True)

            # pre-warm ACT function tables (a table load mid-kernel costs
            # ~1.3us). Only 3 ACT funcs are used anywhere (Exp, Sqrt, Relu)
            # so the table cache never thrashes.
            warm = small.tile([P, 1], f32, tag="warm")
            for fn in (AF.Exp, AF.Sqrt, AF.Square, AF.Relu):
                nc.scalar.activation(out=warm[:], in_=eps_sb[:], func=fn)

            # ---------------- Phase A: attention ----------------
            with tc.tile_pool(name="phA", bufs=1) as pa:
                xb_sb = pa.tile([P, BL, CT, L], bf16, tag="xb")
                x_sb = pa.tile([P, BL, CT, L], f32, tag="x")
                xT_sb = pa.tile([P, BL, LT, C], bf16, tag="xT")
                wkq_sb = pa.tile([P, CT, 2 * A], bf16, tag="wkq")
                wp_sb = pa.tile([P, CT, C], f8, tag="wp")
                keys_sb = pa.tile([P, BL, L], bf16, tag="keys")
                queries_sb = pa.tile([P, BL, L], bf16, tag="q")
                e_sb = pa.tile([P, BL, LT, L], f8, tag="e")
                xTs = pa.tile([P, BL, LT, C], f8, tag="xTs")
                ao_sb = pa.tile([P, BL, CT, L], f8, tag="ao")
                rsp = pa.tile([P, BL, LT, MC], f32, tag="rsp")
                rcp = pa.tile([P, BL, LT], f32, tag="rcp")

                # startup DMAs in need-order (sync queue is FIFO):
                # wkq -> xb(b0) -> xT(b0) -> xb(b1) -> wp -> x -> xT(b1) -> w1
                nc.sync.dma_start(out=wkq_sb[:],
                                  in_=wkq_ext[:].rearrange("p (ct a) -> p ct a", ct=CT))
                for ct in range(CT):
                    nc.sync.dma_start(out=xb_sb[:, 0, ct, :],
                                      in_=xb_ext[0, ct * P:(ct + 1) * P, :])
                nc.sync.dma_start(out=xT_sb[:, 0],
                                  in_=xT_ext[0].rearrange("(lc p) c -> p lc c", p=P))
                for ct in range(CT):
                    nc.sync.dma_start(out=xb_sb[:, 1, ct, :],
                                      in_=xb_ext[1, ct * P:(ct + 1) * P, :])
                nc.sync.dma_start(out=wp_sb[:],
                                  in_=wp_ext[:].rearrange("p (ct o) -> p ct o", ct=CT))
                for b in range(BL):
                    for ct in range(CT):
                        nc.sync.dma_start(out=x_sb[:, b, ct, :],
                                          in_=x_ext[b, ct * P:(ct + 1) * P, :])
                nc.sync.dma_start(out=xT_sb[:, 1],
                                  in_=xT_ext[1].rearrange("(lc p) c -> p lc c", p=P))
                nc.sync.dma_start(out=w1_sb[:],
                                  in_=w1_ext[:].rearrange("p (kc c) -> p kc c", c=C))

                def kq(b):
                    for mc in range(MC):
                        ms = slice(mc * 512, (mc + 1) * 512)
                        kps = psum.tile([P, 512], f32, tag="ps")
                        for ct in range(CT):
                            nc.tensor.matmul(
                                out=kps[:],
                                lhsT=wkq_sb[:, ct, :],
                                rhs=xb_sb[:, b, ct, ms],
                                start=(ct == 0), stop=(ct == CT - 1))
                        # rows 0:64 = keys+bk, 64:128 = queries+bq
                        nc.vector.tensor_scalar_add(out=keys_sb[0:A, b, ms],
                                                    in0=kps[0:A, :],
                                                    scalar1=bkq_sb[0:A])
                        nc.vector.tensor_scalar_add(
                            out=keys_sb[A:2 * A, b, ms],
                            in0=kps[A:2 * A, :], scalar1=bkq_sb[A:2 * A])
                        # move queries down to partition base 0 (SBUF->SBUF)
                        nc.gpsimd.dma_start(out=queries_sb[0:A, b, ms],
                                            in_=keys_sb[A:2 * A, b, ms])

                def scores_lc(b, lc):
                    for mc in range(MC):
                        sps = psum.tile([P, 512], f32, tag="ps")
                        nc.tensor.matmul(
                            out=sps[:],
                            lhsT=keys_sb[0:A, b, lc * P:(lc + 1) * P],
                            rhs=queries_sb[0:A, b, mc * 512:(mc + 1) * 512],
                            start=True, stop=True)
                        # accum_out gives the softmax row-sum for free (no
                        # DVE reduce, no extra serial hop)
                        nc.scalar.activation(
                            out=e_sb[:, b, lc, mc * 512:(mc + 1) * 512],
                            in_=sps[:], func=AF.Exp, scale=SM_SCALE,
                            accum_out=rsp[:, b, lc, mc:mc + 1])
                    nc.vector.scalar_tensor_tensor(
                        out=rcp[:, b, lc:lc + 1], in0=rsp[:, b, lc, 0:1],
                        scalar=1.0, in1=rsp[:, b, lc, 1:2],
                        op0=ALU.mult, op1=ALU.add)
                    nc.vector.reciprocal(out=rcp[:, b, lc:lc + 1],
                                         in_=rcp[:, b, lc:lc + 1])
                    # xTs[l, c] = xT[l, c] / rowsum[l] (softmax denom folded)
                    nc.vector.tensor_scalar_mul(out=xTs[:, b, lc, :],
                                                in0=xT_sb[:, b, lc, :],
                                                scalar1=rcp[:, b, lc:lc + 1])

                def attnout_group(b, cc, mc):
                    # fp8 DoubleRow: two lc-tiles (K=256) per matmul
                    ms = slice(mc * 512, (mc + 1) * 512)
                    aps = psum.tile([P, 512], f32, tag="ps")
                    for lcp in range(0, LT, 2):
                        nc.tensor.matmul(
                            out=aps[:],
                            lhsT=xTs[:, b, lcp:lcp + 2, cc * P:(cc + 1) * P],
                            rhs=e_sb[:, b, lcp:lcp + 2, ms],
                            start=(lcp == 0), stop=(lcp == LT - 2),
                            perf_mode=DR)
                    # ao = attnout (undo the 256x xT host-scale); DVE copy
                    # keeps the in-order ACT queue free for exps
                    nc.vector.tensor_scalar_mul(out=ao_sb[:, b, cc, ms],
                                                in0=aps[:], scalar1=1.0 / 256.0)

                def proj_group(b, oc, mc):
                    ms = slice(mc * 512, (mc + 1) * 512)
                    pps = psum.tile([P, 512], f32, tag="ps")
                    for ctp in range(0, CT, 2):
                        nc.tensor.matmul(
                            out=pps[:],
                            lhsT=wp_sb[:, ctp:ctp + 2, oc * P:(oc + 1) * P],
                            rhs=ao_sb[:, b, ctp:ctp + 2, ms],
                            start=(ctp == 0), stop=(ctp == CT - 2),
                            perf_mode=DR)
                    # x2 = proj + bp + x ; accum_out = per-chunk channel sums
                    nc.vector.scalar_tensor_tensor(
                        out=x2_sb[:, b, oc, ms], in0=pps[:],
                        scalar=bp_sb[:, oc:oc + 1],
                        in1=x_sb[:, b, oc, ms],
                        op0=ALU.add, op1=ALU.add,
                        accum_out=m1a[:, oc, 2 * b + mc:2 * b + mc + 1])
                    # sum(x2^2) for BN1 var on ACT (DVE is the proj-phase
                    # bottleneck; ACT is idle once the exps drain)
                    sqs = ostage.tile([P, 512], f32, tag="sqs")
                    nc.scalar.activation(
                        out=sqs[:], in_=x2_sb[:, b, oc, ms], func=AF.Square,
                        accum_out=m2a[:, oc, 2 * b + mc:2 * b + mc + 1])

                # schedule: b1's scores interleave with b0's attnout groups
                # (the exp chain is ACT-throughput-bound; interleaving paces
                # PSUM allocation to exp consumption)
                filler(8)
                kq(0)
                filler(4)
                for lc in range(LT):
                    scores_lc(0, lc)
                kq(1)
                filler(18)
                g = 0
                for cc in range(CT):
                    for mc in range(MC):
                        attnout_group(0, cc, mc)
                        if g < LT:
                            scores_lc(1, g)
                            g += 1
                filler(2)
                for mc in range(MC):
                    for oc in range(CT):
                        proj_group(0, oc, mc)
                filler(24)
                for cc in range(CT):
                    for mc in range(MC):
                        attnout_group(1, cc, mc)
                filler(2)
                for mc in range(MC):
                    for oc in range(CT):
                        proj_group(1, oc, mc)
                        if mc == 1:
                            # eager per-oc stat pack + AllReduce-input DMA
                            # (hides the ~5.5us HBM-write completion latency)
                            nc.vector.tensor_reduce(
                                out=ccin1_sb[:, oc, 0:1], in_=m1a[:, oc, :],
                                axis=mybir.AxisListType.X, op=ALU.add)
                            nc.vector.tensor_reduce(
                                out=ccin1_sb[:, oc, 1:2], in_=m2a[:, oc, :],
                                axis=mybir.AxisListType.X, op=ALU.add)
                            nc.sync.dma_start(
                                out=cc1_in[:, 2 * oc:2 * oc + 2],
                                in_=ccin1_sb[:, oc, :])

            def stats_allreduce(ccin_dram, ccout_dram, ccred_sb):
                # input bounce DMAs are issued eagerly per-oc by the caller;
                # doorbell + result read on gpsimd
                nc.gpsimd.collective_compute(
                    "AllReduce", mybir.AluOpType.add, replica_groups=rg,
                    ins=[ccin_dram[:].opt()], outs=[ccout_dram[:].opt()])
                nc.gpsimd.dma_start(out=ccred_sb[:], in_=ccout_dram[:])

            def bn_post(ccout_sb, g_sb, be_sb, scale_t, bias_t, tag):
                mgx = small.tile([P, CT, 2], f32, tag=tag + "mgx")
                nc.vector.tensor_scalar_mul(out=mgx[:], in0=ccout_sb[:],
                                            scalar1=1.0 / (B * L))
                mg = mgx[:, :, 0]
                ex2 = mgx[:, :, 1]
                nvar = small.tile([P, CT], f32, tag=tag + "nv")
                # nvar = mean^2 - E[x^2] = -var
                nc.vector.tensor_tensor(out=nvar[:], in0=mg, in1=mg, op=ALU.mult)
                nc.vector.tensor_tensor(out=nvar[:], in0=nvar[:], in1=ex2,
                                        op=ALU.subtract)
                sd = small.tile([P, CT], f32, tag=tag + "sd")
                nc.scalar.activation(out=sd[:], in_=nvar[:], func=AF.Sqrt,
                                     scale=-1.0, bias=eps_sb[:])
                rstd = small.tile([P, CT], f32, tag=tag + "rstd")
                nc.vector.reciprocal(out=rstd[:], in_=sd[:])
                nc.vector.tensor_tensor(out=scale_t[:], in0=rstd[:], in1=g_sb[:],
                                        op=ALU.mult)
                tmp = small.tile([P, CT], f32, tag=tag + "tmp")
                nc.vector.tensor_tensor(out=tmp[:], in0=mg, in1=scale_t[:],
                                        op=ALU.mult)
                nc.vector.tensor_tensor(out=bias_t[:], in0=be_sb[:], in1=tmp[:],
                                        op=ALU.subtract)

            stats_allreduce(cc1_in, cc1_out, ccout1_sb)
            # w2 load queued on gpsimd AFTER the AR1 result DMA: never
            # contends with the collective, done long before conv2.

            # ---------------- Phase B: BN + convs ----------------
            with tc.tile_pool(name="phB", bufs=1) as pb:
                h_sb = pb.tile([P, BL, CT, L + 2], bf16, tag="hpad")
                h2_sb = pb.tile([P, BL, CT, L], f32, tag="h2")
                w2_sb = pb.tile([P, 3 * CT, C], bf16, tag="w2")

                nc.gpsimd.dma_start(
                    out=w2_sb[:],
                    in_=w2_ext[:].rearrange("p (kc c) -> p kc c", c=C))

                # pad zeros (cols 0 and L+1) via DVE memset (keeps the ACT
                # table cache at 3 functions)
                nc.vector.memset(h_sb[:, :, :, 0], 0.0)
                nc.vector.memset(h_sb[:, :, :, L + 1], 0.0)

                bn_post(ccout1_sb, g1_sb, be1_sb, scale1, bias1, "p1")
                # re-warm the PE during the h-relu window (runnable only
                # once bn_post lands, i.e. right after the AllReduce)
                nc.vector.tensor_copy(out=scrw[:, 0:CT], in_=scale1[:])
                filler(12)

                def h_relu(src_sb, scale_t, bias_t):
                    # h chunks ordered to match conv group order:
                    # (b0 half0 ct0..3), (b0 half1), (b1 half0), (b1 half1)
                    for b in range(BL):
                        for half in range(2):
                            for ct in range(CT):
                                if half == 0:
                                    o = slice(1, 1 + H1)
                                    i = slice(0, H1)
                                else:
                                    o = slice(1 + H1, L + 1)
                                    i = slice(H1, L)
                                nc.scalar.activation(
                                    out=h_sb[:, b, ct, o],
                                    in_=src_sb[:, b, ct, i], func=AF.Relu,
                                    scale=scale_t[:, ct:ct + 1],
                                    bias=bias_t[:, ct:ct + 1])

                h_relu(x2_sb, scale1, bias1)

                # conv1: h2[o, l] = sum_{ct,k} w1[k][i,o].T @ h[i, l+k-1] + b1
                # un-interleaved groups: 12 consecutive matmuls per bank
                for oc in range(CT):
                    for b in range(BL):
                        for hc in range(MC):
                            cps = psum.tile([P, 512], f32, tag="ps")
                            for ct in range(CT):
                                for k in range(3):
                                    nc.tensor.matmul(
                                        out=cps[:],
                                        lhsT=w1_sb[:, k * CT + ct,
                                                   oc * P:(oc + 1) * P],
                                        rhs=h_sb[:, b, ct,
                                                 hc * 512 + k:hc * 512 + k + 512],
                                        start=(ct == 0 and k == 0),
                                        stop=(ct == CT - 1 and k == 2))
                            hs = slice(hc * 512, (hc + 1) * 512)
                            nc.vector.tensor_scalar(
                                out=h2_sb[:, b, oc, hs],
                                in0=cps[:], scalar1=b1_sb[:, oc:oc + 1],
                                scalar2=0.0, op0=ALU.add, op1=ALU.add,
                                accum_out=n1a[:, oc, 2 * b + hc:2 * b + hc + 1])
                            sqs = ostage.tile([P, 512], f32, tag="sqs")
                            nc.scalar.activation(
                                out=sqs[:], in_=h2_sb[:, b, oc, hs],
                                func=AF.Square,
                                accum_out=n2a[:, oc, 2 * b + hc:2 * b + hc + 1])
                    # eager per-oc stat pack + AllReduce-input DMA
                    nc.vector.tensor_reduce(
                        out=ccin2_sb[:, oc, 0:1], in_=n1a[:, oc, :],
                        axis=mybir.AxisListType.X, op=ALU.add)
                    nc.vector.tensor_reduce(
                        out=ccin2_sb[:, oc, 1:2], in_=n2a[:, oc, :],
                        axis=mybir.AxisListType.X, op=ALU.add)
                    nc.sync.dma_start(out=cc2_in[:, 2 * oc:2 * oc + 2],
                                      in_=ccin2_sb[:, oc, :])

                stats_allreduce(cc2_in, cc2_out, ccout2_sb)
                bn_post(ccout2_sb, g2_sb, be2_sb, scale2, bias2, "p2")
                nc.vector.tensor_copy(out=scrw[:, 0:CT], in_=scale2[:])
                filler(12)

                # h3 = relu(bn2(h2)) overwrites h_sb in place (pads kept)
                h_relu(h2_sb, scale2, bias2)

                # conv2 + b2 + residual(x2) -> out, stored per 512-chunk
                for oc in range(CT):
                    for b in range(BL):
                        for hc in range(MC):
                            cps = psum.tile([P, 512], f32, tag="ps")
                            for ct in range(CT):
                                for k in range(3):
                                    nc.tensor.matmul(
                                        out=cps[:],
                                        lhsT=w2_sb[:, k * CT + ct,
                                                   oc * P:(oc + 1) * P],
                                        rhs=h_sb[:, b, ct,
                                                 hc * 512 + k:hc * 512 + k + 512],
                                        start=(ct == 0 and k == 0),
                                        stop=(ct == CT - 1 and k == 2))
                            hs = slice(hc * 512, (hc + 1) * 512)
                            og = ostage.tile([P, 512], f32, tag="og")
                            nc.vector.scalar_tensor_tensor(
                                out=og[:], in0=cps[:],
                                scalar=b2_sb[:, oc:oc + 1],
                                in1=x2_sb[:, b, oc, hs],
                                op0=ALU.add, op1=ALU.add)
                            nc.sync.dma_start(
                                out=out_ext[b, oc * P:(oc + 1) * P, hs],
                                in_=og[:])

    nc.compile()
    return nc


def _get_nc():
    if "nc" not in _CACHE:
        _CACHE["nc"] = _build()
    return _CACHE["nc"]


def _prep_in_maps(inputs):
    import ml_dtypes
    f = np.float32
    bf = ml_dtypes.bfloat16
    x = np.ascontiguousarray(inputs["x"], dtype=f)
    def vec_pct(v):
        # (C,) -> [P, CT] with channel c = ct*P + p at [p, ct]
        return np.asarray(v, dtype=f).reshape(CT, P).T
    pvec = np.concatenate(
        [vec_pct(inputs[k]) for k in ("bp", "b1", "b2", "g1", "be1", "g2", "be2")]
        + [np.concatenate([inputs["bk"], inputs["bq"]]).reshape(P, 1).astype(f)],
        axis=1)
    def swiz2(w):  # [C, X] -> [P, CT*X] partition-major
        X = w.shape[1]
        return np.ascontiguousarray(
            w.reshape(CT, P, X).transpose(1, 0, 2).reshape(P, CT * X))
    def swiz3(w):  # [3, C, C] (k, i, o) -> [P, 3*CT*C] with cols (k*CT+ct)*C+o
        return np.ascontiguousarray(
            w.reshape(3, CT, P, C).transpose(2, 0, 1, 3).reshape(P, 3 * CT * C))
    f8 = ml_dtypes.float8_e4m3
    shared = {
        "wkq": swiz2(np.concatenate([inputs["Wk"].T, inputs["Wq"].T],
                                    axis=1).astype(bf)),
        "wp": swiz2(inputs["Wp"].T.astype(f8)),
        "w1": swiz3(np.transpose(inputs["W1"], (2, 1, 0)).astype(bf)),
        "w2": swiz3(np.transpose(inputs["W2"], (2, 1, 0)).astype(bf)),
        "pvec": np.ascontiguousarray(pvec, dtype=f),
    }
    in_maps = []
    for i in range(NCORES):
        xl = np.ascontiguousarray(x[i * BL:(i + 1) * BL])
        # xT scaled x256 so xTs = xT/rowsum lands in fp8-normal range;
        # the 1/256 is undone at the attnout PSUM->SBUF copy
        xTl = np.ascontiguousarray(
            (np.transpose(xl, (0, 2, 1)) * 256.0).astype(bf))
        m = {"x": xl, "xb": xl.astype(bf), "xT": xTl}
        m.update(shared)
        in_maps.append(m)
    return in_maps


def kernel(**inputs) -> np.ndarray:
    from concourse import bass_utils
    nc = _get_nc()
    in_maps = _prep_in_maps(inputs)
    res = bass_utils.run_bass_kernel_spmd(nc, in_maps, list(range(NCORES)))
    return np.concatenate([r["out"] for r in res.results], axis=0)


# revision 35
# speedup vs baseline: 1.0778x; 1.0778x over previous
"""Trainium2 Bass kernel for nn_AttnResBlock (B=16, C=512, A=64, L=1024).

Data-parallel over batch: 8 cores x 2 batches each, weights replicated.
BatchNorm (training mode, stats over (B, L)) needs global batch stats ->
two tiny [128, 8] f32 AllReduces; a same-shape warmup collective at kernel
start absorbs the ~50us first-collective setup under phase A.

v2 schedule (vs baseline):
  - kq projections in bf16 (host-cast copy of x); x stays f32 for the
    residual only. xT host-supplied in bf16 (feeds xTs directly).
  - fine-grained startup DMAs: wkq + xb(b0) first -> first matmul ~6us.
  - phase A interleaved across the 2 local batches to kill the
    batch-transition bubble: kq(b0,b1) | scores b0 | attnout b0 x
    scores b1 | proj b0 | attnout b1 | proj b1.
  - BN stats reduced eagerly per channel-tile; sum-of-squares on ACT
    (idle during proj/conv) so the AllReduce fires ~1us after the last
    proj/conv matmul.
  - w1 loaded during phase A; w2 on the gpsimd queue right after AR1's
    result DMA (never contends with a collective).
  - h = relu(bn(x)) written in half-L chunks ordered to match the conv
    matmul groups, so conv starts ~2us after the AllReduce lands.
  - conv accumulation groups un-interleaved (12 consecutive matmuls per
    PSUM bank); outputs stored per 512-chunk to shrink the tail.
"""
import numpy as np

P = 128
B, C, A, L = 16, 512, 64, 1024
NCORES = 8
BL = B // NCORES          # local batches per core
CT = C // P               # 4 channel tiles
LT = L // P               # 8 length tiles
MC = L // 512             # 2 moving chunks
EPS = 1e-5
SM_SCALE = 2.0 / L        # softmax scale: scores/(L/2)
H1 = 514                  # first h chunk covers cols 0:514 (taps for hc=0)

_CACHE = {}


def _build():
    import concourse.bass as bass
    import concourse.mybir as mybir
    from concourse import bacc
    from concourse.tile import TileContext

    f32 = mybir.dt.float32
    bf16 = mybir.dt.bfloat16
    f8 = mybir.dt.float8e4
    DR = mybir.MatmulPerfMode.DoubleRow
    AF = mybir.ActivationFunctionType
    ALU = mybir.AluOpType

    nc = bacc.Bacc(num_devices=NCORES)

    x_ext = nc.declare_dram_parameter("x", [BL, C, L], f32, isOutput=False)
    xb_ext = nc.declare_dram_parameter("xb", [BL, C, L], bf16, isOutput=False)
    xT_ext = nc.declare_dram_parameter("xT", [BL, L, C], bf16, isOutput=False)
    wkq_ext = nc.declare_dram_parameter("wkq", [P, CT * 2 * A], bf16, isOutput=False)
    wp_ext = nc.declare_dram_parameter("wp", [P, CT * C], f8, isOutput=False)
    w1_ext = nc.declare_dram_parameter("w1", [P, 3 * CT * C], bf16, isOutput=False)
    w2_ext = nc.declare_dram_parameter("w2", [P, 3 * CT * C], bf16, isOutput=False)
    # per-channel vectors packed [P, CT] each: bp b1 b2 g1 be1 g2 be2, then bkq
    pvec_ext = nc.declare_dram_parameter("pvec", [P, 7 * CT + 1], f32, isOutput=False)
    out_ext = nc.declare_dram_parameter("out", [BL, C, L], f32, isOutput=True)

    cc0_in = nc.dram_tensor("cc0_in", [P, 2 * CT], f32)
    cc0_out = nc.dram_tensor("cc0_out", [P, 2 * CT], f32, addr_space="Shared")
    cc1_in = nc.dram_tensor("cc1_in", [P, 2 * CT], f32)
    cc1_out = nc.dram_tensor("cc1_out", [P, 2 * CT], f32, addr_space="Shared")
    cc2_in = nc.dram_tensor("cc2_in", [P, 2 * CT], f32)
    cc2_out = nc.dram_tensor("cc2_out", [P, 2 * CT], f32, addr_space="Shared")

    rg = [list(range(NCORES))]

    with TileContext(nc) as tc:
        with tc.tile_pool(name="persist", bufs=1) as pers, \
             tc.tile_pool(name="small", bufs=1) as small, \
             tc.tile_pool(name="ostage", bufs=4) as ostage, \
             tc.tile_pool(name="psum", bufs=7, space="PSUM") as psum, \
             tc.tile_pool(name="psumf", bufs=1, space="PSUM") as psumf:

            x2_sb = pers.tile([P, BL, CT, L], f32)
            w1_sb = pers.tile([P, 3 * CT, C], bf16, tag="w1")

            # warmup collective: absorbs the first-collective setup cost
            # (~50us barrier+init) under phase A
            nc.gpsimd.collective_compute(
                "AllReduce", mybir.AluOpType.add, replica_groups=rg,
                ins=[cc0_in[:].opt()], outs=[cc0_out[:].opt()])

            pvec_sb = small.tile([P, 7 * CT + 1], f32, tag="pvec")
            nc.gpsimd.dma_start(out=pvec_sb[:], in_=pvec_ext[:])
            bp_sb = pvec_sb[:, 0 * CT:1 * CT]
            b1_sb = pvec_sb[:, 1 * CT:2 * CT]
            b2_sb = pvec_sb[:, 2 * CT:3 * CT]
            g1_sb = pvec_sb[:, 3 * CT:4 * CT]
            be1_sb = pvec_sb[:, 4 * CT:5 * CT]
            g2_sb = pvec_sb[:, 5 * CT:6 * CT]
            be2_sb = pvec_sb[:, 6 * CT:7 * CT]
            bkq_sb = pvec_sb[:, 7 * CT:7 * CT + 1]   # [bk; bq]

            # stat layout [P, oc, (sum, sumsq)] so each oc's pair is adjacent
            # (enables eager per-oc DMA of AllReduce input)
            ccin1_sb = small.tile([P, CT, 2], f32, tag="ccin1")
            ccout1_sb = small.tile([P, CT, 2], f32, tag="ccout1")
            ccin2_sb = small.tile([P, CT, 2], f32, tag="ccin2")
            ccout2_sb = small.tile([P, CT, 2], f32, tag="ccout2")
            # per-chunk stat accumulators: [P, ct, 2*b+chunk]
            m1a = small.tile([P, CT, 2 * BL], f32, tag="m1a")   # sum(x2)
            m2a = small.tile([P, CT, 2 * BL], f32, tag="m2a")   # sum(x2^2)
            n1a = small.tile([P, CT, 2 * BL], f32, tag="n1a")   # sum(h2)
            n2a = small.tile([P, CT, 2 * BL], f32, tag="n2a")   # sum(h2^2)
            scale1 = small.tile([P, CT], f32, tag="scale1")
            bias1 = small.tile([P, CT], f32, tag="bias1")
            scale2 = small.tile([P, CT], f32, tag="scale2")
            bias2 = small.tile([P, CT], f32, tag="bias2")
            eps_sb = small.tile([P, 1], f32, tag="eps")
            nc.vector.memset(eps_sb[:], EPS)
            # scratch for post-AllReduce PE re-warm fillers (written after
            # bn_post so the fillers become runnable just before the convs)
            scrw = small.tile([P, 640], bf16, tag="scrw")
            nc.vector.memset(scrw[:], 0.0)
            fillps = psumf.tile([P, 512], f32, tag="fill")

            def filler(n):
                # dummy matmuls with no data deps: keep HAM at 8/8 through
                # dependency bubbles (a >=3.4us PE idle re-throttles the PE
                # clock to 1.2GHz for 10-30us)
                for _ in range(n):
                    nc.tensor.matmul(out=fillps[:], lhsT=scrw[:, 0:128],
                                     rhs=scrw[:, 128:640], start=True,
                                     stop=True)

            # pre-warm ACT function tables (a table load mid-kernel costs
            # ~1.3us). Only 3 ACT funcs are used anywhere (Exp, Sqrt, Relu)
            # so the table cache never thrashes.
            warm = small.tile([P, 1], f32, tag="warm")
            for fn in (AF.Exp, AF.Sqrt, AF.Square, AF.Relu):
                nc.scalar.activation(out=warm[:], in_=eps_sb[:], func=fn)

            # ---------------- Phase A: attention ----------------
            with tc.tile_pool(name="phA", bufs=1) as pa:
                xb_sb = pa.tile([P, BL, CT, L], bf16, tag="xb")
                x_sb = pa.tile([P, BL, CT, L], f32, tag="x")
                xT_sb = pa.tile([P, BL, LT, C], bf16, tag="xT")
                wkq_sb = pa.tile([P, CT, 2 * A], bf16, tag="wkq")
                wp_sb = pa.tile([P, CT, C], f8, tag="wp")
                keys_sb = pa.tile([P, BL, L], bf16, tag="keys")
                queries_sb = pa.tile([P, BL, L], bf16, tag="q")
                e_sb = pa.tile([P, BL, LT, L], f8, tag="e")
                xTs = pa.tile([P, BL, LT, C], f8, tag="xTs")
                ao_sb = pa.tile([P, BL, CT, L], f8, tag="ao")
                rsp = pa.tile([P, BL, LT, MC], f32, tag="rsp")
                rcp = pa.tile([P, BL, LT], f32, tag="rcp")

                # startup DMAs in need-order (sync queue is FIFO):
                # wkq -> xb(b0) -> xT(b0) -> xb(b1) -> wp -> x -> xT(b1) -> w1
                nc.sync.dma_start(out=wkq_sb[:],
                                  in_=wkq_ext[:].rearrange("p (ct a) -> p ct a", ct=CT))
                for ct in range(CT):
                    nc.sync.dma_start(out=xb_sb[:, 0, ct, :],
                                      in_=xb_ext[0, ct * P:(ct + 1) * P, :])
                nc.sync.dma_start(out=xT_sb[:, 0],
                                  in_=xT_ext[0].rearrange("(lc p) c -> p lc c", p=P))
                for ct in range(CT):
                    nc.sync.dma_start(out=xb_sb[:, 1, ct, :],
                                      in_=xb_ext[1, ct * P:(ct + 1) * P, :])
                nc.sync.dma_start(out=wp_sb[:],
                                  in_=wp_ext[:].rearrange("p (ct o) -> p ct o", ct=CT))
                for b in range(BL):
                    for ct in range(CT):
                        nc.sync.dma_start(out=x_sb[:, b, ct, :],
                                          in_=x_ext[b, ct * P:(ct + 1) * P, :])
                nc.sync.dma_start(out=xT_sb[:, 1],
                                  in_=xT_ext[1].rearrange("(lc p) c -> p lc c", p=P))
                nc.sync.dma_start(out=w1_sb[:],
                                  in_=w1_ext[:].rearrange("p (kc c) -> p kc c", c=C))

                def kq(b):
                    for mc in range(MC):
                        ms = slice(mc * 512, (mc + 1) * 512)
                        kps = psum.tile([P, 512], f32, tag="ps")
                        for ct in range(CT):
                            nc.tensor.matmul(
                                out=kps[:],
                                lhsT=wkq_sb[:, ct, :],
                                rhs=xb_sb[:, b, ct, ms],
                                start=(ct == 0), stop=(ct == CT - 1))
                        # rows 0:64 = keys+bk, 64:128 = queries+bq
                        nc.vector.tensor_scalar_add(out=keys_sb[0:A, b, ms],
                                                    in0=kps[0:A, :],
                                                    scalar1=bkq_sb[0:A])
                        nc.vector.tensor_scalar_add(
                            out=keys_sb[A:2 * A, b, ms],
                            in0=kps[A:2 * A, :], scalar1=bkq_sb[A:2 * A])
                        # move queries down to partition base 0 (SBUF->SBUF)
                        nc.gpsimd.dma_start(out=queries_sb[0:A, b, ms],
                                            in_=keys_sb[A:2 * A, b, ms])

                def scores_lc(b, lc):
                    for mc in range(MC):
                        sps = psum.tile([P, 512], f32, tag="ps")
                        nc.tensor.matmul(
                            out=sps[:],
                            lhsT=keys_sb[0:A, b, lc * P:(lc + 1) * P],
                            rhs=queries_sb[0:A, b, mc * 512:(mc + 1) * 512],
                            start=True, stop=True)
                        # accum_out gives the softmax row-sum for free (no
                        # DVE reduce, no extra serial hop)
                        nc.scalar.activation(
                            out=e_sb[:, b, lc, mc * 512:(mc + 1) * 512],
                            in_=sps[:], func=AF.Exp, scale=SM_SCALE,
                            accum_out=rsp[:, b, lc, mc:mc + 1])
                    nc.vector.scalar_tensor_tensor(
                        out=rcp[:, b, lc:lc + 1], in0=rsp[:, b, lc, 0:1],
                        scalar=1.0, in1=rsp[:, b, lc, 1:2],
                        op0=ALU.mult, op1=ALU.add)
                    nc.vector.reciprocal(out=rcp[:, b, lc:lc + 1],
                                         in_=rcp[:, b, lc:lc + 1])
                    # xTs[l, c] = xT[l, c] / rowsum[l] (softmax denom folded)
                    nc.vector.tensor_scalar_mul(out=xTs[:, b, lc, :],
                                                in0=xT_sb[:, b, lc, :],
                                                scalar1=rcp[:, b, lc:lc + 1])

                def attnout_group(b, cc, mc):
                    # fp8 DoubleRow: two lc-tiles (K=256) per matmul
                    ms = slice(mc * 512, (mc + 1) * 512)
                    aps = psum.tile([P, 512], f32, tag="ps")
                    for lcp in range(0, LT, 2):
                        nc.tensor.matmul(
                            out=aps[:],
                            lhsT=xTs[:, b, lcp:lcp + 2, cc * P:(cc + 1) * P],
                            rhs=e_sb[:, b, lcp:lcp + 2, ms],
                            start=(lcp == 0), stop=(lcp == LT - 2),
                            perf_mode=DR)
                    # ao = attnout (undo the 256x xT host-scale); DVE copy
                    # keeps the in-order ACT queue free for exps
                    nc.vector.tensor_scalar_mul(out=ao_sb[:, b, cc, ms],
                                                in0=aps[:], scalar1=1.0 / 256.0)

                def proj_group(b, oc, mc):
                    ms = slice(mc * 512, (mc + 1) * 512)
                    pps = psum.tile([P, 512], f32, tag="ps")
                    for ctp in range(0, CT, 2):
                        nc.tensor.matmul(
                            out=pps[:],
                            lhsT=wp_sb[:, ctp:ctp + 2, oc * P:(oc + 1) * P],
                            rhs=ao_sb[:, b, ctp:ctp + 2, ms],
                            start=(ctp == 0), stop=(ctp == CT - 2),
                            perf_mode=DR)
                    # x2 = proj + bp + x ; accum_out = per-chunk channel sums
                    nc.vector.scalar_tensor_tensor(
                        out=x2_sb[:, b, oc, ms], in0=pps[:],
                        scalar=bp_sb[:, oc:oc + 1],
                        in1=x_sb[:, b, oc, ms],
                        op0=ALU.add, op1=ALU.add,
                        accum_out=m1a[:, oc, 2 * b + mc:2 * b + mc + 1])
                    # sum(x2^2) for BN1 var on ACT (DVE is the proj-phase
                    # bottleneck; ACT is idle once the exps drain)
                    sqs = ostage.tile([P, 512], f32, tag="sqs")
                    nc.scalar.activation(
                        out=sqs[:], in_=x2_sb[:, b, oc, ms], func=AF.Square,
                        accum_out=m2a[:, oc, 2 * b + mc:2 * b + mc + 1])

                # schedule: b1's scores interleave with b0's attnout groups
                # (the exp chain is ACT-throughput-bound; interleaving paces
                # PSUM allocation to exp consumption)
                filler(8)
                kq(0)
                filler(4)
                for lc in range(LT):
                    scores_lc(0, lc)
                kq(1)
                filler(18)
                g = 0
                for cc in range(CT):
                    for mc in range(MC):
                        attnout_group(0, cc, mc)
                        if g < LT:
                            scores_lc(1, g)
                            g += 1
                filler(2)
                for mc in range(MC):
                    for oc in range(CT):
                        proj_group(0, oc, mc)
                filler(36)
                for cc in range(CT):
                    for mc in range(MC):
                        attnout_group(1, cc, mc)
                filler(2)
                for mc in range(MC):
                    for oc in range(CT):
                        proj_group(1, oc, mc)
                        if mc == 1:
                            # eager per-oc stat pack + AllReduce-input DMA
                            # (hides the ~5.5us HBM-write completion latency)
                            nc.vector.tensor_reduce(
                                out=ccin1_sb[:, oc, 0:1], in_=m1a[:, oc, :],
                                axis=mybir.AxisListType.X, op=ALU.add)
                            nc.vector.tensor_reduce(
                                out=ccin1_sb[:, oc, 1:2], in_=m2a[:, oc, :],
                                axis=mybir.AxisListType.X, op=ALU.add)
                            nc.sync.dma_start(
                                out=cc1_in[:, 2 * oc:2 * oc + 2],
                                in_=ccin1_sb[:, oc, :])

            def stats_allreduce(ccin_dram, ccout_dram, ccred_sb):
                # input bounce DMAs are issued eagerly per-oc by the caller;
                # doorbell + result read on gpsimd
                nc.gpsimd.collective_compute(
                    "AllReduce", mybir.AluOpType.add, replica_groups=rg,
                    ins=[ccin_dram[:].opt()], outs=[ccout_dram[:].opt()])
                nc.gpsimd.dma_start(out=ccred_sb[:], in_=ccout_dram[:])

            def bn_post(ccout_sb, g_sb, be_sb, scale_t, bias_t, tag):
                mgx = small.tile([P, CT, 2], f32, tag=tag + "mgx")
                nc.vector.tensor_scalar_mul(out=mgx[:], in0=ccout_sb[:],
                                            scalar1=1.0 / (B * L))
                mg = mgx[:, :, 0]
                ex2 = mgx[:, :, 1]
                nvar = small.tile([P, CT], f32, tag=tag + "nv")
                # nvar = mean^2 - E[x^2] = -var
                nc.vector.tensor_tensor(out=nvar[:], in0=mg, in1=mg, op=ALU.mult)
                nc.vector.tensor_tensor(out=nvar[:], in0=nvar[:], in1=ex2,
                                        op=ALU.subtract)
                sd = small.tile([P, CT], f32, tag=tag + "sd")
                nc.scalar.activation(out=sd[:], in_=nvar[:], func=AF.Sqrt,
                                     scale=-1.0, bias=eps_sb[:])
                rstd = small.tile([P, CT], f32, tag=tag + "rstd")
                nc.vector.reciprocal(out=rstd[:], in_=sd[:])
                nc.vector.tensor_tensor(out=scale_t[:], in0=rstd[:], in1=g_sb[:],
                                        op=ALU.mult)
                tmp = small.tile([P, CT], f32, tag=tag + "tmp")
                nc.vector.tensor_tensor(out=tmp[:], in0=mg, in1=scale_t[:],
                                        op=ALU.mult)
                nc.vector.tensor_tensor(out=bias_t[:], in0=be_sb[:], in1=tmp[:],
                                        op=ALU.subtract)

            stats_allreduce(cc1_in, cc1_out, ccout1_sb)
            # w2 load queued on gpsimd AFTER the AR1 result DMA: never
            # contends with the collective, done long before conv2.

            # ---------------- Phase B: BN + convs ----------------
            with tc.tile_pool(name="phB", bufs=1) as pb:
                h_sb = pb.tile([P, BL, CT, L + 2], bf16, tag="hpad")
                h2_sb = pb.tile([P, BL, CT, L], f32, tag="h2")
                w2_sb = pb.tile([P, 3 * CT, C], bf16, tag="w2")

                nc.gpsimd.dma_start(
                    out=w2_sb[:],
                    in_=w2_ext[:].rearrange("p (kc c) -> p kc c", c=C))

                # pad zeros (cols 0 and L+1) via DVE memset (keeps the ACT
                # table cache at 3 functions)
                nc.vector.memset(h_sb[:, :, :, 0], 0.0)
                nc.vector.memset(h_sb[:, :, :, L + 1], 0.0)

                bn_post(ccout1_sb, g1_sb, be1_sb, scale1, bias1, "p1")
                # re-warm the PE during the h-relu window (runnable only
                # once bn_post lands, i.e. right after the AllReduce)
                nc.vector.tensor_copy(out=scrw[:, 0:CT], in_=scale1[:])
                filler(12)

                def h_relu(src_sb, scale_t, bias_t):
                    # h chunks ordered to match conv group order:
                    # (b0 half0 ct0..3), (b0 half1), (b1 half0), (b1 half1)
                    for b in range(BL):
                        for half in range(2):
                            for ct in range(CT):
                                if half == 0:
                                    o = slice(1, 1 + H1)
                                    i = slice(0, H1)
                                else:
                                    o = slice(1 + H1, L + 1)
                                    i = slice(H1, L)
                                nc.scalar.activation(
                                    out=h_sb[:, b, ct, o],
                                    in_=src_sb[:, b, ct, i], func=AF.Relu,
                                    scale=scale_t[:, ct:ct + 1],
                                    bias=bias_t[:, ct:ct + 1])

                h_relu(x2_sb, scale1, bias1)

                # conv1: h2[o, l] = sum_{ct,k} w1[k][i,o].T @ h[i, l+k-1] + b1
                # un-interleaved groups: 12 consecutive matmuls per bank
                for oc in range(CT):
                    for b in range(BL):
                        for hc in range(MC):
                            cps = psum.tile([P, 512], f32, tag="ps")
                            for ct in range(CT):
                                for k in range(3):
                                    nc.tensor.matmul(
                                        out=cps[:],
                                        lhsT=w1_sb[:, k * CT + ct,
                                                   oc * P:(oc + 1) * P],
                                        rhs=h_sb[:, b, ct,
                                                 hc * 512 + k:hc * 512 + k + 512],
                                        start=(ct == 0 and k == 0),
                                        stop=(ct == CT - 1 and k == 2))
                            hs = slice(hc * 512, (hc + 1) * 512)
                            nc.vector.tensor_scalar(
                                out=h2_sb[:, b, oc, hs],
                                in0=cps[:], scalar1=b1_sb[:, oc:oc + 1],
                                scalar2=0.0, op0=ALU.add, op1=ALU.add,
                                accum_out=n1a[:, oc, 2 * b + hc:2 * b + hc + 1])
                            sqs = ostage.tile([P, 512], f32, tag="sqs")
                            nc.scalar.activation(
                                out=sqs[:], in_=h2_sb[:, b, oc, hs],
                                func=AF.Square,
                                accum_out=n2a[:, oc, 2 * b + hc:2 * b + hc + 1])
                    # eager per-oc stat pack + AllReduce-input DMA
                    nc.vector.tensor_reduce(
                        out=ccin2_sb[:, oc, 0:1], in_=n1a[:, oc, :],
                        axis=mybir.AxisListType.X, op=ALU.add)
                    nc.vector.tensor_reduce(
                        out=ccin2_sb[:, oc, 1:2], in_=n2a[:, oc, :],
                        axis=mybir.AxisListType.X, op=ALU.add)
                    nc.sync.dma_start(out=cc2_in[:, 2 * oc:2 * oc + 2],
                                      in_=ccin2_sb[:, oc, :])

                stats_allreduce(cc2_in, cc2_out, ccout2_sb)
                bn_post(ccout2_sb, g2_sb, be2_sb, scale2, bias2, "p2")
                nc.vector.tensor_copy(out=scrw[:, 0:CT], in_=scale2[:])
                filler(12)

                # h3 = relu(bn2(h2)) overwrites h_sb in place (pads kept)
                h_relu(h2_sb, scale2, bias2)

                # conv2 + b2 + residual(x2) -> out, stored per 512-chunk
                for oc in range(CT):
                    for b in range(BL):
                        for hc in range(MC):
                            cps = psum.tile([P, 512], f32, tag="ps")
                            for ct in range(CT):
                                for k in range(3):
                                    nc.tensor.matmul(
                                        out=cps[:],
                                        lhsT=w2_sb[:, k * CT + ct,
                                                   oc * P:(oc + 1) * P],
                                        rhs=h_sb[:, b, ct,
                                                 hc * 512 + k:hc * 512 + k + 512],
                                        start=(ct == 0 and k == 0),
                                        stop=(ct == CT - 1 and k == 2))
                            hs = slice(hc * 512, (hc + 1) * 512)
                            og = ostage.tile([P, 512], f32, tag="og")
                            nc.vector.scalar_tensor_tensor(
                                out=og[:], in0=cps[:],
                                scalar=b2_sb[:, oc:oc + 1],
                                in1=x2_sb[:, b, oc, hs],
                                op0=ALU.add, op1=ALU.add)
                            nc.sync.dma_start(
                                out=out_ext[b, oc * P:(oc + 1) * P, hs],
                                in_=og[:])

    nc.compile()
    return nc


def _get_nc():
    if "nc" not in _CACHE:
        _CACHE["nc"] = _build()
    return _CACHE["nc"]


def _prep_in_maps(inputs):
    import ml_dtypes
    f = np.float32
    bf = ml_dtypes.bfloat16
    x = np.ascontiguousarray(inputs["x"], dtype=f)
    def vec_pct(v):
        # (C,) -> [P, CT] with channel c = ct*P + p at [p, ct]
        return np.asarray(v, dtype=f).reshape(CT, P).T
    pvec = np.concatenate(
        [vec_pct(inputs[k]) for k in ("bp", "b1", "b2", "g1", "be1", "g2", "be2")]
        + [np.concatenate([inputs["bk"], inputs["bq"]]).reshape(P, 1).astype(f)],
        axis=1)
    def swiz2(w):  # [C, X] -> [P, CT*X] partition-major
        X = w.shape[1]
        return np.ascontiguousarray(
            w.reshape(CT, P, X).transpose(1, 0, 2).reshape(P, CT * X))
    def swiz3(w):  # [3, C, C] (k, i, o) -> [P, 3*CT*C] with cols (k*CT+ct)*C+o
        return np.ascontiguousarray(
            w.reshape(3, CT, P, C).transpose(2, 0, 1, 3).reshape(P, 3 * CT * C))
    f8 = ml_dtypes.float8_e4m3
    shared = {
        "wkq": swiz2(np.concatenate([inputs["Wk"].T, inputs["Wq"].T],
                                    axis=1).astype(bf)),
        "wp": swiz2(inputs["Wp"].T.astype(f8)),
        "w1": swiz3(np.transpose(inputs["W1"], (2, 1, 0)).astype(bf)),
        "w2": swiz3(np.transpose(inputs["W2"], (2, 1, 0)).astype(bf)),
        "pvec": np.ascontiguousarray(pvec, dtype=f),
    }
    in_maps = []
    for i in range(NCORES):
        xl = np.ascontiguousarray(x[i * BL:(i + 1) * BL])
        # xT scaled x256 so xTs = xT/rowsum lands in fp8-normal range;
        # the 1/256 is undone at the attnout PSUM->SBUF copy
        xTl = np.ascontiguousarray(
            (np.transpose(xl, (0, 2, 1)) * 256.0).astype(bf))
        m = {"x": xl, "xb": xl.astype(bf), "xT": xTl}
        m.update(shared)
        in_maps.append(m)
    return in_maps


def kernel(**inputs) -> np.ndarray:
    from concourse import bass_utils
    nc = _get_nc()
    in_maps = _prep_in_maps(inputs)
    res = bass_utils.run_bass_kernel_spmd(nc, in_maps, list(range(NCORES)))
    return np.concatenate([r["out"] for r in res.results], axis=0)
